# revision 31
# baseline (speedup 1.0000x reference)
"""Trainium2 Bass kernel for nn_Classifier_48223892799748 (retrieval_knn).

Computes sim = (D + enc_pm @ cent_pm.T) / 2 where
  enc_pm = sign((samples - 0.5) @ weight.T)  in {+1,-1}
  cent_pm = centroids mapped {0,1} -> {-1,+1}

Sharding: data-parallel over the batch dim (8192 -> 1024 rows per core,
8 cores). weight / centroids replicated.

Device layout: everything is computed transposed ([D, B] / [C, B]) so the
matmul-1 output tile [128 d, 512 b] feeds matmul-2 directly as the moving
operand (contraction over d) with no on-device transpose.

Both matmuls run as fp8e4m3 DoubleRow (256-deep contraction per pass,
2x the f32r/bf16 MAC rate):
  matmul-1: samples quantized to fp8 on host. All on-device arithmetic is
            exact on the fp8 lattice (f32 PSUM accumulation of fp8
            products), so the end-to-end result equals the host-side
            numpy simulation bit-for-bit: rel err ~8.9e-3 purely from
            host quantization sign flips.
  matmul-2: exact (+/-0.5 x +/-1 products, f32 accumulation).

Thresholding runs on DVE as (x > 0) - 0.5 -> {+0.5, -0.5} in fp8 (one
[128, 1024] tensor_scalar per d-tile). The HW ACT Sign activation returns
NaN for the exact-zero PSUM values the fp8 lattice produces, so Sign is
avoided entirely; the 2x encoding scale is absorbed into the final copy
(scale 1.0 instead of 0.5).
"""

import sys

if "/opt/trn_rl_repo" not in sys.path:
    sys.path.insert(0, "/opt/trn_rl_repo")

import ml_dtypes
import numpy as np

import concourse.bass as bass
import concourse.mybir as mybir
import concourse.tile as tile
from concourse import bacc
from concourse.bass_utils import run_bass_kernel_spmd

# The container's `antenv` package is a stub without `axon_hooks`; if tracing
# is ever requested (BASS_TRACE=1), run_bass_kernel_spmd imports it and would
# crash. Provide a stub module (hook=None -> tracing skipped gracefully)
# unless something (e.g. a test harness) registered a real one already.
try:  # pragma: no cover
    import antenv.axon_hooks  # noqa: F401
except ImportError:
    import types as _types

    import antenv as _antenv

    _hooks = _types.ModuleType("antenv.axon_hooks")
    _hook_store = {"h": None}
    _hooks.set_axon_ntff_profile_hook = lambda h: _hook_store.__setitem__("h", h)
    _hooks.get_axon_ntff_profile_hook = lambda: _hook_store["h"]
    sys.modules["antenv.axon_hooks"] = _hooks
    _antenv.axon_hooks = _hooks

FP8 = ml_dtypes.float8_e4m3

B, IN_F, D, C = 8192, 1024, 10000, 100
N_CORES = 8
B_SH = B // N_CORES          # 1024 batch rows per core
KCP = IN_F // 256            # 4 DoubleRow contraction pairs for matmul 1
DT = (D + 127) // 128        # 79 d-tiles
D_PAD = DT * 128             # 10112
NB = B_SH // 512             # 2 psum-bank-width chunks of the local batch
NPAIR = (DT + 1) // 2        # 40 d-tile pairs for DoubleRow matmul-2
D_PAD2 = NPAIR * 256         # 10240
C_PAD = 112                  # DoubleRow weight AP needs byte-step %16 == 0
CENTER = 0.5

# Stash of the last BassKernelResults (exec_time_ns etc.) for test harnesses.
LAST_RUN = None
_NC_CACHE = None


def _build_nc():
    nc = bacc.Bacc("TRN2", target_bir_lowering=False)
    f32 = mybir.dt.float32
    fp8 = mybir.dt.float8e4
    COPY = mybir.ActivationFunctionType.Copy
    DR = mybir.MatmulPerfMode.DoubleRow

    # DRAM I/O (per-core shard layouts, see host prep in kernel()):
    #   s8:  [128 p, KCP, 2, B_SH] fp8   s8[p,t,j,b] = fp8(samples[b, (2t+j)*128+p] - 0.5)
    #   w8:  [128 p, DT, KCP, 2, 128 d]  w8[p,dt,t,j,d] = W[dt*128+d, (2t+j)*128+p]
    #   ct:  [128 p, NPAIR, 2, C_PAD]    ct[p,t,j,c] = cent_pm[c, t*256+j*128+p]
    #   out: [C, 2, 512] uint16          2*sim.T shard (exact: 2*sim =
    #                                    agree + 10000 in [0, 20480];
    #                                    halves the result-DMA bytes)
    u16 = mybir.dt.uint16
    s8_d = nc.dram_tensor("s8", [128, KCP, 2, B_SH], fp8, kind="ExternalInput")
    w8_d = nc.dram_tensor("w8", [128, DT, KCP, 2, 128], fp8, kind="ExternalInput")
    ct_d = nc.dram_tensor("ct", [128, NPAIR, 2, C_PAD], fp8, kind="ExternalInput")
    out_d = nc.dram_tensor("out", [C, NB, 512], u16, kind="ExternalOutput")

    with tile.TileContext(nc) as tc:
        with (
            tc.tile_pool(name="const", bufs=1) as const_pool,
            tc.tile_pool(name="wts", bufs=4) as w_pool,
            tc.tile_pool(name="enc", bufs=6) as enc_pool,
            tc.tile_pool(name="outp", bufs=1) as out_pool,
            tc.tile_pool(name="ps1", bufs=3, space=bass.MemorySpace.PSUM) as ps1_pool,
            tc.tile_pool(name="ps2", bufs=1, space=bass.MemorySpace.PSUM) as ps2_pool,
        ):
            s8 = const_pool.tile([128, KCP, 2, B_SH], fp8)
            cent = const_pool.tile([128, NPAIR, 2, C_PAD], fp8)
            # The DMA-completion semaphore pool is ~9 wide and re-arming a
            # semaphore waits on its previous consumers, so the early phase
            # can only keep a couple of small transfers in flight. Batch the
            # first WB weight tiles into one const buffer loaded via a few
            # large sub-slice triggers (few semaphores, full bandwidth); the
            # per-dt pool DMAs start at dt=WB once the pool has drained.
            WB = 8
            wbig = const_pool.tile([128, WB, KCP, 2, 128], fp8)
            # fast start: trigger issue on Sync costs ~650 ns apiece, so
            # order the preamble DMAs by first-need time. The first matmul
            # only needs w00 (32 KB) + the (t=0, b=0) sample chunk (128 KB).
            w00 = const_pool.tile([128, 2, 128], fp8)
            # Serial trigger issue on ONE engine doubles as a priority
            # scheduler: each transfer mostly completes before the next
            # bulk one competes for DMA engines, so keep everything on
            # Sync in first-need order. (A fully parallel multi-engine
            # issue measured ~4 us WORSE: the bulk prefetch competed with
            # the critical first 300 KB.) Only w00 — the first matmul's
            # stationary — issues from the otherwise-idle Scalar engine,
            # in parallel with Sync's first sample trigger.
            nc.scalar.dma_start(w00[:], w8_d[:, 0, 0, :, :])
            nc.sync.dma_start(s8[:, 0, :, 0:512], s8_d[:, 0, :, 0:512])
            nc.sync.dma_start(s8[:, 0, :, 512:B_SH], s8_d[:, 0, :, 512:B_SH])
            nc.sync.dma_start(wbig[:, 0], w8_d[:, 0])
            nc.sync.dma_start(s8[:, 1, :, :], s8_d[:, 1, :, :])
            nc.sync.dma_start(wbig[:, 1:3], w8_d[:, 1:3])
            nc.sync.dma_start(s8[:, 2, :, :], s8_d[:, 2, :, :])
            nc.sync.dma_start(s8[:, 3, :, :], s8_d[:, 3, :, :])
            nc.sync.dma_start(wbig[:, 3:WB], w8_d[:, 3:WB])
            nc.sync.dma_start(cent[:], ct_d[:])

            ps2 = ps2_pool.tile([C_PAD, NB, 512], f32, name="ps2")

            # PE p-state warmup: the clock ramps 0.65 -> 1.2 -> 2.4 GHz only
            # after ~3 us of continuous busy, and the PE would otherwise sit
            # idle until the first weight/sample DMAs land (~10 us). Run a
            # block of throwaway DR matmuls on memset scratch during that
            # window so the real stream starts near full clock. Results land
            # in ps2, which the real pair-0 matmul resets via start=True.
            warm_w = const_pool.tile([128, 2, C_PAD], fp8)
            warm_s = const_pool.tile([128, 2, 256], fp8)
            nc.gpsimd.memset(warm_w[:], 0.0)
            nc.gpsimd.memset(warm_s[:], 0.0)

            def warm_mm(n):
                for _ in range(n):
                    nc.tensor.matmul(
                        ps2[:, 0, 0:256],
                        warm_w[:],
                        warm_s[:],
                        start=True,
                        stop=True,
                        perf_mode=DR,
                    )

            warm_mm(12)

            # software pipeline: matmul2 for pair t0 is issued on PE a few
            # pairs late, so PE never waits on the threshold round-trip; the
            # first flush is held back further so the centroid load (1.1 MB)
            # never gates early PE progress.
            pending = []

            def flush_pending():
                t0, enc_c = pending.pop(0)
                for b in range(NB):
                    nc.tensor.matmul(
                        ps2[:, b, :],
                        cent[:, t0, :, :],
                        enc_c[:, :, b, :],
                        start=(t0 == 0),
                        stop=(t0 == NPAIR - 1),
                        perf_mode=DR,
                    )

            cur_pair = None
            for dt in range(DT):
                if dt >= WB:
                    # one trigger per 128 KB tile: the runtime shards each
                    # transfer into ~1.6 KB packets across all 16 DMA
                    # engines, so one trigger still gets full bandwidth
                    w = w_pool.tile([128, KCP, 2, 128], fp8, tag="w", name=f"w_{dt}")
                    nc.sync.dma_start(w[:], w8_d[:, dt])
                ps1 = ps1_pool.tile([128, NB, 512], f32, tag="ps1", name=f"ps1_{dt}")
                for t in range(KCP):
                    if dt == 0 and t == 0:
                        w_src = w00[:]
                    elif dt < WB:
                        w_src = wbig[:, dt, t, :, :]
                    else:
                        w_src = w[:, t, :, :]
                    for b in range(NB):
                        nc.tensor.matmul(
                            ps1[:, b, :],
                            w_src,
                            s8[:, t, :, bass.ts(b, 512)],
                            start=(t == 0),
                            stop=(t == KCP - 1),
                            perf_mode=DR,
                        )
                    if dt == 0 and t <= 1:
                        # keep the p-state ramp alive through the early
                        # data-bound stalls: these fillers execute exactly
                        # when the PE would otherwise idle waiting on the
                        # next sample/weight transfer (in-order engine),
                        # and cost at most their own length if data is
                        # already resident
                        warm_mm(3 if t == 0 else 2)
                j = dt % 2
                if j == 0:
                    cur_pair = enc_pool.tile(
                        [128, 2, NB, 512], fp8, tag="enc", name=f"e_{dt}"
                    )
                # threshold both b-chunks in one op: (ps1 > 0) - 0.5
                nc.vector.tensor_scalar(
                    out=cur_pair[:, j, :, :],
                    in0=ps1[:, :, :],
                    scalar1=0.0,
                    scalar2=0.5,
                    op0=mybir.AluOpType.is_gt,
                    op1=mybir.AluOpType.subtract,
                )
                if dt == DT - 1 and j == 0:
                    # phantom j=1 half of the final pair (dt=79 doesn't
                    # exist): zero it so 0-weight x garbage can't poison PSUM
                    nc.gpsimd.memset(cur_pair[:, 1, :, :], 0.0)
                if j == 1 or dt == DT - 1:
                    pending.append((dt // 2, cur_pair))
                if len(pending) >= (4 if dt < 20 else 2):
                    flush_pending()
            while pending:
                flush_pending()

            ob = out_pool.tile([C, NB, 512], u16, name="ob")
            # +/-0.5 encodings make ps2 = agree/2, so 2*ps2 + D = 2*sim —
            # an exact integer in [0, 20480] that fits uint16 (host halves
            # it back to f32). The out DMA is issued from Scalar right
            # behind its own copy (in-order, no cross-engine hop).
            nc.scalar.activation(ob[:], ps2[:C, :, :], COPY, bias=float(D), scale=2.0)
            nc.scalar.dma_start(out_d[:], ob[:])

    nc.compile()
    return nc


def _get_nc():
    global _NC_CACHE
    if _NC_CACHE is None:
        _NC_CACHE = _build_nc()
    return _NC_CACHE


def kernel(samples, weight, centroids):
    global LAST_RUN
    samples = np.asarray(samples, dtype=np.float32)
    weight = np.asarray(weight, dtype=np.float32)
    centroids = np.asarray(centroids)

    # ---- host-side marshalling (layout + dtype only) ----
    # centered samples, transposed to [IN_F, B], quantized to fp8e4m3
    scT8 = (samples - np.float32(CENTER)).T.astype(FP8)

    def s_core(c):
        # [IN_F, B_SH] -> [128 p, KCP, 2, B_SH]
        blk = scT8[:, c * B_SH : (c + 1) * B_SH]
        return np.ascontiguousarray(
            blk.reshape(KCP, 2, 128, B_SH).transpose(2, 0, 1, 3)
        )

    # weight.T DoubleRow tiles: w8[p, dt, t, j, d] = W[dt*128+d, (2t+j)*128+p]
    wpad = np.zeros((D_PAD, IN_F), dtype=np.float32)
    wpad[:D] = weight  # +/-1, exact in fp8
    w8 = np.ascontiguousarray(
        wpad.reshape(DT, 128, KCP, 2, 128).transpose(4, 0, 2, 3, 1).astype(FP8)
    )

    # DoubleRow centroid tiles: ct[p, t, j, c] = cent_pm[c, t*256+j*128+p]
    cpad = np.zeros((D_PAD2, C_PAD), dtype=np.float32)
    cpad[:D, :C] = np.where(centroids, np.float32(1.0), np.float32(-1.0)).T
    ct = np.ascontiguousarray(
        cpad.reshape(NPAIR, 2, 128, C_PAD).transpose(2, 0, 1, 3).astype(FP8)
    )

    in_maps = [{"s8": s_core(c), "w8": w8, "ct": ct} for c in range(N_CORES)]

    nc = _get_nc()
    res = run_bass_kernel_spmd(nc, in_maps, core_ids=list(range(N_CORES)))
    LAST_RUN = res

    # gather: out[c] is 2*sim.T ([C, B_SH] uint16) for batch rows
    # [c*B_SH, (c+1)*B_SH); halve back to the exact f32 match counts
    return np.vstack(
        [
            np.asarray(res.results[c]["out"]).reshape(C, B_SH).T
            for c in range(N_CORES)
        ]
    ).astype(np.float32) * np.float32(0.5)


# revision 32
# speedup vs baseline: 1.0157x; 1.0157x over previous
"""Trainium2 Bass kernel for nn_Classifier_48223892799748 (retrieval_knn).

Computes sim = (D + enc_pm @ cent_pm.T) / 2 where
  enc_pm = sign((samples - 0.5) @ weight.T)  in {+1,-1}
  cent_pm = centroids mapped {0,1} -> {-1,+1}

Sharding: data-parallel over the batch dim (8192 -> 1024 rows per core,
8 cores). weight / centroids replicated.

Device layout: everything is computed transposed ([D, B] / [C, B]) so the
matmul-1 output tile [128 d, 512 b] feeds matmul-2 directly as the moving
operand (contraction over d) with no on-device transpose.

Both matmuls run as fp8e4m3 DoubleRow (256-deep contraction per pass,
2x the f32r/bf16 MAC rate):
  matmul-1: samples quantized to fp8 on host. All on-device arithmetic is
            exact on the fp8 lattice (f32 PSUM accumulation of fp8
            products), so the end-to-end result equals the host-side
            numpy simulation bit-for-bit: rel err ~8.9e-3 purely from
            host quantization sign flips.
  matmul-2: exact (+/-0.5 x +/-1 products, f32 accumulation).

Thresholding runs on DVE as (x > 0) - 0.5 -> {+0.5, -0.5} in fp8 (one
[128, 1024] tensor_scalar per d-tile). The HW ACT Sign activation returns
NaN for the exact-zero PSUM values the fp8 lattice produces, so Sign is
avoided entirely; the 2x encoding scale is absorbed into the final copy
(scale 1.0 instead of 0.5).
"""

import sys

if "/opt/trn_rl_repo" not in sys.path:
    sys.path.insert(0, "/opt/trn_rl_repo")

import ml_dtypes
import numpy as np

import concourse.bass as bass
import concourse.mybir as mybir
import concourse.tile as tile
from concourse import bacc
from concourse.bass_utils import run_bass_kernel_spmd

# The container's `antenv` package is a stub without `axon_hooks`; if tracing
# is ever requested (BASS_TRACE=1), run_bass_kernel_spmd imports it and would
# crash. Provide a stub module (hook=None -> tracing skipped gracefully)
# unless something (e.g. a test harness) registered a real one already.
try:  # pragma: no cover
    import antenv.axon_hooks  # noqa: F401
except ImportError:
    import types as _types

    import antenv as _antenv

    _hooks = _types.ModuleType("antenv.axon_hooks")
    _hook_store = {"h": None}
    _hooks.set_axon_ntff_profile_hook = lambda h: _hook_store.__setitem__("h", h)
    _hooks.get_axon_ntff_profile_hook = lambda: _hook_store["h"]
    sys.modules["antenv.axon_hooks"] = _hooks
    _antenv.axon_hooks = _hooks

FP8 = ml_dtypes.float8_e4m3

B, IN_F, D, C = 8192, 1024, 10000, 100
N_CORES = 8
B_SH = B // N_CORES          # 1024 batch rows per core
KCP = IN_F // 256            # 4 DoubleRow contraction pairs for matmul 1
DT = (D + 127) // 128        # 79 d-tiles
D_PAD = DT * 128             # 10112
NB = B_SH // 512             # 2 psum-bank-width chunks of the local batch
NPAIR = (DT + 1) // 2        # 40 d-tile pairs for DoubleRow matmul-2
D_PAD2 = NPAIR * 256         # 10240
C_PAD = 112                  # DoubleRow weight AP needs byte-step %16 == 0
CENTER = 0.5

# Stash of the last BassKernelResults (exec_time_ns etc.) for test harnesses.
LAST_RUN = None
_NC_CACHE = None


def _build_nc():
    nc = bacc.Bacc("TRN2", target_bir_lowering=False)
    f32 = mybir.dt.float32
    fp8 = mybir.dt.float8e4
    COPY = mybir.ActivationFunctionType.Copy
    DR = mybir.MatmulPerfMode.DoubleRow

    # DRAM I/O (per-core shard layouts, see host prep in kernel()):
    #   s8:  [128 p, KCP, 2, B_SH] fp8   s8[p,t,j,b] = fp8(samples[b, (2t+j)*128+p] - 0.5)
    #   w8:  [128 p, DT, KCP, 2, 128 d]  w8[p,dt,t,j,d] = W[dt*128+d, (2t+j)*128+p]
    #   ct:  [128 p, NPAIR, 2, C_PAD]    ct[p,t,j,c] = cent_pm[c, t*256+j*128+p]
    #   out: [C, 2, 512] uint16          2*sim.T shard (exact: 2*sim =
    #                                    agree + 10000 in [0, 20480];
    #                                    halves the result-DMA bytes)
    u16 = mybir.dt.uint16
    s8_d = nc.dram_tensor("s8", [128, KCP, 2, B_SH], fp8, kind="ExternalInput")
    w8_d = nc.dram_tensor("w8", [128, DT, KCP, 2, 128], fp8, kind="ExternalInput")
    ct_d = nc.dram_tensor("ct", [128, NPAIR, 2, C_PAD], fp8, kind="ExternalInput")
    out_d = nc.dram_tensor("out", [C, NB, 512], u16, kind="ExternalOutput")

    with tile.TileContext(nc) as tc:
        with (
            tc.tile_pool(name="const", bufs=1) as const_pool,
            tc.tile_pool(name="wts", bufs=4) as w_pool,
            tc.tile_pool(name="enc", bufs=6) as enc_pool,
            tc.tile_pool(name="outp", bufs=1) as out_pool,
            tc.tile_pool(name="ps1", bufs=3, space=bass.MemorySpace.PSUM) as ps1_pool,
            tc.tile_pool(name="ps2", bufs=1, space=bass.MemorySpace.PSUM) as ps2_pool,
        ):
            s8 = const_pool.tile([128, KCP, 2, B_SH], fp8)
            cent = const_pool.tile([128, NPAIR, 2, C_PAD], fp8)
            # The DMA-completion semaphore pool is ~9 wide and re-arming a
            # semaphore waits on its previous consumers, so the early phase
            # can only keep a couple of small transfers in flight. Batch the
            # first WB weight tiles into one const buffer loaded via a few
            # large sub-slice triggers (few semaphores, full bandwidth); the
            # per-dt pool DMAs start at dt=WB once the pool has drained.
            WB = 8
            wbig = const_pool.tile([128, WB, KCP, 2, 128], fp8)
            # fast start: trigger issue on Sync costs ~650 ns apiece, so
            # order the preamble DMAs by first-need time. The first matmul
            # only needs w00 (32 KB) + the (t=0, b=0) sample chunk (128 KB).
            w00 = const_pool.tile([128, 2, 128], fp8)
            # Serial trigger issue on ONE engine doubles as a priority
            # scheduler: each transfer mostly completes before the next
            # bulk one competes for DMA engines, so keep everything on
            # Sync in first-need order. (A fully parallel multi-engine
            # issue measured ~4 us WORSE: the bulk prefetch competed with
            # the critical first 300 KB.) Only w00 — the first matmul's
            # stationary — issues from the otherwise-idle Scalar engine,
            # in parallel with Sync's first sample trigger.
            nc.scalar.dma_start(w00[:], w8_d[:, 0, 0, :, :])
            nc.sync.dma_start(s8[:, 0, :, 0:512], s8_d[:, 0, :, 0:512])
            nc.sync.dma_start(s8[:, 0, :, 512:B_SH], s8_d[:, 0, :, 512:B_SH])
            nc.sync.dma_start(wbig[:, 0], w8_d[:, 0])
            nc.sync.dma_start(s8[:, 1, :, :], s8_d[:, 1, :, :])
            nc.sync.dma_start(wbig[:, 1:3], w8_d[:, 1:3])
            nc.sync.dma_start(s8[:, 2, :, :], s8_d[:, 2, :, :])
            nc.sync.dma_start(s8[:, 3, :, :], s8_d[:, 3, :, :])
            nc.sync.dma_start(wbig[:, 3:WB], w8_d[:, 3:WB])
            nc.sync.dma_start(cent[:], ct_d[:])

            ps2 = ps2_pool.tile([C_PAD, NB, 512], f32, name="ps2")

            # PE p-state warmup: the clock ramps 0.65 -> 1.2 -> 2.4 GHz only
            # after ~3 us of continuous busy, and the PE would otherwise sit
            # idle until the first weight/sample DMAs land (~10 us). Run a
            # block of throwaway DR matmuls on memset scratch during that
            # window so the real stream starts near full clock. Results land
            # in ps2, which the real pair-0 matmul resets via start=True.
            warm_w = const_pool.tile([128, 2, C_PAD], fp8)
            warm_s = const_pool.tile([128, 2, 256], fp8)
            nc.gpsimd.memset(warm_w[:], 0.0)
            nc.gpsimd.memset(warm_s[:], 0.0)
            for _ in range(10):
                nc.tensor.matmul(
                    ps2[:, 0, 0:256],
                    warm_w[:],
                    warm_s[:],
                    start=True,
                    stop=True,
                    perf_mode=DR,
                )

            # software pipeline: matmul2 for pair t0 is issued on PE a few
            # pairs late, so PE never waits on the threshold round-trip; the
            # first flush is held back further so the centroid load (1.1 MB)
            # never gates early PE progress.
            pending = []

            def flush_pending():
                t0, enc_c = pending.pop(0)
                for b in range(NB):
                    nc.tensor.matmul(
                        ps2[:, b, :],
                        cent[:, t0, :, :],
                        enc_c[:, :, b, :],
                        start=(t0 == 0),
                        stop=(t0 == NPAIR - 1),
                        perf_mode=DR,
                    )

            cur_pair = None
            for dt in range(DT):
                if dt >= WB:
                    # one trigger per 128 KB tile: the runtime shards each
                    # transfer into ~1.6 KB packets across all 16 DMA
                    # engines, so one trigger still gets full bandwidth
                    w = w_pool.tile([128, KCP, 2, 128], fp8, tag="w", name=f"w_{dt}")
                    nc.sync.dma_start(w[:], w8_d[:, dt])
                ps1 = ps1_pool.tile([128, NB, 512], f32, tag="ps1", name=f"ps1_{dt}")
                for t in range(KCP):
                    if dt == 0 and t == 0:
                        w_src = w00[:]
                    elif dt < WB:
                        w_src = wbig[:, dt, t, :, :]
                    else:
                        w_src = w[:, t, :, :]
                    for b in range(NB):
                        nc.tensor.matmul(
                            ps1[:, b, :],
                            w_src,
                            s8[:, t, :, bass.ts(b, 512)],
                            start=(t == 0),
                            stop=(t == KCP - 1),
                            perf_mode=DR,
                        )
                j = dt % 2
                if j == 0:
                    cur_pair = enc_pool.tile(
                        [128, 2, NB, 512], fp8, tag="enc", name=f"e_{dt}"
                    )
                # threshold both b-chunks in one op: (ps1 > 0) - 0.5
                nc.vector.tensor_scalar(
                    out=cur_pair[:, j, :, :],
                    in0=ps1[:, :, :],
                    scalar1=0.0,
                    scalar2=0.5,
                    op0=mybir.AluOpType.is_gt,
                    op1=mybir.AluOpType.subtract,
                )
                if dt == DT - 1 and j == 0:
                    # phantom j=1 half of the final pair (dt=79 doesn't
                    # exist): zero it so 0-weight x garbage can't poison PSUM
                    nc.gpsimd.memset(cur_pair[:, 1, :, :], 0.0)
                if j == 1 or dt == DT - 1:
                    pending.append((dt // 2, cur_pair))
                if len(pending) >= (4 if dt < 20 else 2):
                    flush_pending()
            while pending:
                flush_pending()

            ob = out_pool.tile([C, NB, 512], u16, name="ob")
            # +/-0.5 encodings make ps2 = agree/2, so 2*ps2 + D = 2*sim —
            # an exact integer in [0, 20480] that fits uint16 (host halves
            # it back to f32). The out DMA is issued from Scalar right
            # behind its own copy (in-order, no cross-engine hop).
            nc.scalar.activation(ob[:], ps2[:C, :, :], COPY, bias=float(D), scale=2.0)
            nc.scalar.dma_start(out_d[:], ob[:])

    nc.compile()
    return nc


def _get_nc():
    global _NC_CACHE
    if _NC_CACHE is None:
        _NC_CACHE = _build_nc()
    return _NC_CACHE


def kernel(samples, weight, centroids):
    global LAST_RUN
    samples = np.asarray(samples, dtype=np.float32)
    weight = np.asarray(weight, dtype=np.float32)
    centroids = np.asarray(centroids)

    # ---- host-side marshalling (layout + dtype only) ----
    # centered samples, transposed to [IN_F, B], quantized to fp8e4m3
    scT8 = (samples - np.float32(CENTER)).T.astype(FP8)

    def s_core(c):
        # [IN_F, B_SH] -> [128 p, KCP, 2, B_SH]
        blk = scT8[:, c * B_SH : (c + 1) * B_SH]
        return np.ascontiguousarray(
            blk.reshape(KCP, 2, 128, B_SH).transpose(2, 0, 1, 3)
        )

    # weight.T DoubleRow tiles: w8[p, dt, t, j, d] = W[dt*128+d, (2t+j)*128+p]
    wpad = np.zeros((D_PAD, IN_F), dtype=np.float32)
    wpad[:D] = weight  # +/-1, exact in fp8
    w8 = np.ascontiguousarray(
        wpad.reshape(DT, 128, KCP, 2, 128).transpose(4, 0, 2, 3, 1).astype(FP8)
    )

    # DoubleRow centroid tiles: ct[p, t, j, c] = cent_pm[c, t*256+j*128+p]
    cpad = np.zeros((D_PAD2, C_PAD), dtype=np.float32)
    cpad[:D, :C] = np.where(centroids, np.float32(1.0), np.float32(-1.0)).T
    ct = np.ascontiguousarray(
        cpad.reshape(NPAIR, 2, 128, C_PAD).transpose(2, 0, 1, 3).astype(FP8)
    )

    in_maps = [{"s8": s_core(c), "w8": w8, "ct": ct} for c in range(N_CORES)]

    nc = _get_nc()
    res = run_bass_kernel_spmd(nc, in_maps, core_ids=list(range(N_CORES)))
    LAST_RUN = res

    # gather: out[c] is 2*sim.T ([C, B_SH] uint16) for batch rows
    # [c*B_SH, (c+1)*B_SH); halve back to the exact f32 match counts
    return np.vstack(
        [
            np.asarray(res.results[c]["out"]).reshape(C, B_SH).T
            for c in range(N_CORES)
        ]
    ).astype(np.float32) * np.float32(0.5)
